# revision 31
# baseline (speedup 1.0000x reference)
"""Distributed single-head attention on 8 TRN2 NeuronCores.

Reference computation (fp32):
    qh = q @ Wq.T ; kh = k @ Wk.T ; vh = v @ Wv.T          [B,S,512]
    scores = (qh @ kh.T) * sqrt(4096)                       [B,S,S]
    scores = where(mask==0, -1e9, scores)
    out = softmax(scores, -1) @ vh                          [B,S,512]
with B=4, S=2048, HIDDEN=4096, HEAD=512.

Sharding: 8 cores = (batch b, seq half h); core c handles query rows
[h*1024, (h+1)*1024) of batch b = c//2.  Each core projects only its own
1024 rows of q/k/v; the pair of cores sharing a batch exchanges khT / vh
via an intra-pair AllGather, overlapped with the q projection.

Precision: the softmax is saturated (score std ~1450 after the *64
scale) so the output is ~one-hot @ argmax of the masked scores — score
precision decides correctness.  The q/k projections and QK^T run as
3-pass fp16 hi/lo split matmuls (hi@hi + hi@lo + lo@hi, fp32 PSUM
accumulation): measured 2.2e-7 relative error on silicon — full fp32
quality at 3 PE cycles/row instead of fp32's 4.  The V path only feeds
the post-softmax average, so it runs in bf16 (1 cycle/row, ~3e-3
relative output error).  PE transposes are bit-exact (verified) and put
the contraction dim on partitions for q/k/v and for P^T in PV.
"""

import os
import sys

import numpy as np


def _ensure_path():
    for p in ("/opt/trn_rl_repo", "/opt/pypackages"):
        if os.path.isdir(p) and p not in sys.path:
            sys.path.append(p)


_ensure_path()

from concourse import bacc, masks, tile  # noqa: E402
from concourse import bass_utils  # noqa: E402
from concourse.bass import mybir  # noqa: E402

# S3 upload is unavailable in this container; keep profile artifacts local.
bass_utils.upload_artifacts = lambda tmpdir: tmpdir

F32 = mybir.dt.float32
F16 = mybir.dt.float16
BF16 = mybir.dt.bfloat16

B, S, E, D = 4, 2048, 4096, 512
N_CORES = 8
S_LOC = B * S // N_CORES  # 1024 rows per core
SCALE = float(E) ** 0.5  # 64.0
NEG = -1e9

P = 128
EC = E // P  # 32 contraction chunks for projections
DC = D // P  # 4 head-dim chunks
NT = S // P  # 16 key tiles
ST = S_LOC // P  # 8 query tiles per core

REPLICA_GROUPS = [[0, 1], [2, 3], [4, 5], [6, 7]]

_COMPILED = None


def _build():
    nc = bacc.Bacc("TRN2", target_bir_lowering=False, debug=False, num_devices=N_CORES)

    q_in = nc.dram_tensor("q", [S_LOC, E], F32, kind="ExternalInput").ap()
    k_in = nc.dram_tensor("k", [S_LOC, E], F32, kind="ExternalInput").ap()
    v_in = nc.dram_tensor("v", [S_LOC, E], F32, kind="ExternalInput").ap()
    # W.T hi/lo fp16 pairs packed as [E, 2, D] (host-prepacked)
    wqt = nc.dram_tensor("wqt", [E, 2, D], F16, kind="ExternalInput").ap()
    wkt = nc.dram_tensor("wkt", [E, 2, D], F16, kind="ExternalInput").ap()
    wvt = nc.dram_tensor("wvt", [E, D], BF16, kind="ExternalInput").ap()
    maskf = nc.dram_tensor("maskf", [1, S], F32, kind="ExternalInput").ap()
    out = nc.dram_tensor("out", [S_LOC, D], F32, kind="ExternalOutput").ap()

    # Internal DRAM bounce buffers for the intra-pair AllGathers.
    # khl_loc packs khT hi at [0] and lo at [1] (fp16).
    khl_loc = nc.dram_tensor("khl_loc", [2, D, S_LOC], F16).ap()
    khl_full = nc.dram_tensor("khl_full", [4, D, S_LOC], F16).ap()
    vh_loc = nc.dram_tensor("vh_loc", [S_LOC, D], BF16).ap()
    vh_full = nc.dram_tensor("vh_full", [S, D], BF16).ap()

    with tile.TileContext(nc) as tc:
        with (
            tc.tile_pool(name="const", bufs=1) as const,
            tc.tile_pool(name="big", bufs=1) as big,
            tc.tile_pool(name="io", bufs=3) as io,
            tc.tile_pool(name="attn", bufs=2) as attn,
            tc.tile_pool(name="small", bufs=4) as small,
            tc.tile_pool(name="pacc", bufs=5, space="PSUM") as pacc,
            tc.tile_pool(name="ptst", bufs=2, space="PSUM") as ptst,
            tc.tile_pool(name="ppv", bufs=1, space="PSUM") as ppv,
        ):
            # ---- constants ----
            ident = const.tile([P, P], F32, tag="ident")
            masks.make_identity(nc, ident[:])
            identb = const.tile([P, P], BF16, tag="identb")
            masks.make_identity(nc, identb[:])
            # maskb[p, t] = maskf[t] for all partitions (0-stride broadcast DMA).
            maskb = const.tile([P, S], F32, tag="maskb")
            nc.sync.dma_start(out=maskb[:], in_=maskf[:].to_broadcast((P, S)))

            # persistent per-core tensors (fp16 hi/lo for the score path)
            qht_h = big.tile([P, DC, S_LOC], F16, tag="qht_h")
            qht_l = big.tile([P, DC, S_LOC], F16, tag="qht_l")
            kht_h = big.tile([P, DC, S], F16, tag="kht_h")
            kht_l = big.tile([P, DC, S], F16, tag="kht_l")
            vh = big.tile([P, NT, D], BF16, tag="vh")

            def load_sup(x_in, g, e4, tag, bufs=3):
                """Load a [512 rows x 512 cols] block of x as [128, 4, 512]
                (8 contiguous-row DMAs on SWDGE, 1KB runs)."""
                xs = io.tile([P, 4, 512], F32, tag=tag, name=f"{tag}_{g}_{e4}", bufs=bufs)
                for j in range(4):
                    for hh in range(2):
                        nc.gpsimd.dma_start(
                            out=xs[:, j, hh * 256 : (hh + 1) * 256],
                            in_=x_in[
                                g * 512 + j * P : g * 512 + (j + 1) * P,
                                e4 * 512 + hh * 256 : e4 * 512 + (hh + 1) * 256,
                            ],
                        )
                return xs

            def project_hl(x_in, w_in, sink, xtag, xbufs=3):
                """fp16 hi/lo projection: psum tiles [d 128, s 512] accumulated
                over 32 e-chunks x 3 passes.  One-iteration software pipeline:
                transposes/casts for e run while matmuls for e-1 execute."""
                for g in range(2):
                    accs = [
                        pacc.tile([P, 512], F32, tag="acc", name=f"acc_{g}_{i}")
                        for i in range(4)
                    ]

                    def stage(e):
                        if e % 4 == 0:
                            stage.xs = load_sup(x_in, g, e // 4, xtag, bufs=xbufs)
                        whl = io.tile(
                            [P, 2, D], F16, tag="whl", name=f"whl_{g}_{e}", bufs=4
                        )
                        nc.scalar.dma_start(
                            out=whl[:], in_=w_in[e * P : (e + 1) * P, :, :]
                        )
                        el = (e % 4) * P
                        trh = io.tile([P, 512], F16, tag="xTh")
                        trl = io.tile([P, 512], F16, tag="xTl")
                        for j in range(4):
                            pt = ptst.tile([P, P], F32, tag="tst", name=f"t_{g}_{e}_{j}")
                            nc.tensor.matmul(
                                pt[:], stage.xs[:, j, el : el + P], ident[:],
                                is_transpose=True,
                            )
                            nc.any.tensor_copy(trh[:, j * P : (j + 1) * P], pt[:])
                            nc.vector.scalar_tensor_tensor(
                                out=trl[:, j * P : (j + 1) * P],
                                in0=trh[:, j * P : (j + 1) * P], scalar=-1.0,
                                in1=pt[:],
                                op0=mybir.AluOpType.mult, op1=mybir.AluOpType.add,
                            )
                        return whl, trh, trl

                    def matmuls(e, whl, trh, trl):
                        for d in range(4):
                            for p_i, (w_i, xt16) in enumerate(
                                ((0, trh), (0, trl), (1, trh))
                            ):
                                nc.tensor.matmul(
                                    accs[d][:],
                                    whl[:, w_i, d * P : (d + 1) * P],
                                    xt16[:],
                                    start=(e == 0 and p_i == 0),
                                    stop=(e == EC - 1 and p_i == 2),
                                )

                    prev = None
                    for e in range(EC):
                        cur = stage(e)
                        if prev is not None:
                            matmuls(e - 1, *prev)
                        prev = cur
                    matmuls(EC - 1, *prev)

                    for j in range(4):
                        sink(g, j, accs[j][:])

            def split_hl(ps, hi_ap, lo_ap):
                """Evict fp32 psum into fp16 hi + lo."""
                nc.any.tensor_copy(hi_ap, ps)
                nc.vector.scalar_tensor_tensor(
                    out=lo_ap, in0=hi_ap, scalar=-1.0, in1=ps,
                    op0=mybir.AluOpType.mult, op1=mybir.AluOpType.add,
                )

            # ---- v projection (bf16, software-pipelined) -> DRAM bounce ----
            def v_sink(g, j, ps):
                stg = io.tile([P, 512], BF16, tag="postgb", name=f"vs_{g}_{j}", bufs=2)
                nc.any.tensor_copy(stg[:], ps)
                t0 = (g * 4 + j) * P
                nc.sync.dma_start(out=vh_loc[t0 : t0 + P, :], in_=stg[:])

            for g in range(2):
                vaccs = [
                    pacc.tile([P, 512], F32, tag="acc", name=f"vacc_{g}_{i}")
                    for i in range(4)
                ]

                def vstage(e, g=g):
                    if e % 4 == 0:
                        vstage.xs = load_sup(v_in, g, e // 4, "xsupv")
                        vstage.xb = io.tile(
                            [P, 4, 512], BF16, tag="xbsup",
                            name=f"xb_{g}_{e}", bufs=2,
                        )
                        nc.scalar.copy(vstage.xb[:], vstage.xs[:])
                    w = io.tile([P, D], BF16, tag="wloadb", name=f"wv_{g}_{e}", bufs=4)
                    nc.scalar.dma_start(out=w[:], in_=wvt[e * P : (e + 1) * P, :])
                    el = (e % 4) * P
                    trjs = []
                    for j in range(4):
                        pt = ptst.tile([P, P], BF16, tag="tst", name=f"vt_{g}_{e}_{j}")
                        nc.tensor.matmul(
                            pt[:], vstage.xb[:, j, el : el + P], identb[:],
                            is_transpose=True,
                        )
                        trj = io.tile([P, P], BF16, tag="xTj", name=f"vj_{g}_{e}_{j}", bufs=6)
                        nc.any.tensor_copy(trj[:], pt[:])
                        trjs.append(trj)
                    return w, trjs

                def vmatmuls(e, w, trjs, vaccs=vaccs):
                    for j in range(4):
                        nc.tensor.matmul(
                            vaccs[j][:], trjs[j][:], w[:],
                            start=(e == 0), stop=(e == EC - 1),
                        )

                prev = None
                for e in range(EC):
                    cur = vstage(e)
                    if prev is not None:
                        vmatmuls(e - 1, *prev)
                    prev = cur
                vmatmuls(EC - 1, *prev)
                for j in range(4):
                    v_sink(g, j, vaccs[j][:])

            # vh AllGather fires as soon as vh_loc is written (overlaps k+q).
            nc.gpsimd.collective_compute(
                "AllGather",
                mybir.AluOpType.bypass,
                replica_groups=REPLICA_GROUPS,
                ins=[vh_loc.opt()],
                outs=[vh_full.opt()],
            )

            # ---- k projection -> khT hi/lo -> DRAM bounce ----
            def k_sink(g, d, ps):
                sh = io.tile([P, 512], F16, tag="postgh", name=f"ksh_{g}_{d}", bufs=2)
                sl = io.tile([P, 512], F16, tag="postgl", name=f"ksl_{g}_{d}", bufs=2)
                split_hl(ps, sh[:], sl[:])
                nc.sync.dma_start(
                    out=khl_loc[0, d * P : (d + 1) * P, g * 512 : (g + 1) * 512],
                    in_=sh[:],
                )
                nc.sync.dma_start(
                    out=khl_loc[1, d * P : (d + 1) * P, g * 512 : (g + 1) * 512],
                    in_=sl[:],
                )

            project_hl(k_in, wkt, k_sink, "xsupk")

            nc.gpsimd.collective_compute(
                "AllGather",
                mybir.AluOpType.bypass,
                replica_groups=REPLICA_GROUPS,
                ins=[khl_loc.opt()],
                outs=[khl_full.opt()],
            )

            # ---- q projection -> qht hi/lo (stays in SBUF) ----
            def q_sink(g, d, ps):
                split_hl(
                    ps,
                    qht_h[:, d, g * 512 : (g + 1) * 512],
                    qht_l[:, d, g * 512 : (g + 1) * 512],
                )

            project_hl(q_in, wqt, q_sink, "xsupq", xbufs=2)

            # ---- gather AG results back to SBUF (split DMAs across engines) ----
            # khl_full[h*2 + {0,1}] = rank-h's khT {hi,lo}.
            for h in range(2):
                for d in range(DC):
                    nc.sync.dma_start(
                        out=kht_h[:, d, h * S_LOC : (h + 1) * S_LOC],
                        in_=khl_full[2 * h, d * P : (d + 1) * P, :],
                    )
                    nc.sync.dma_start(
                        out=kht_l[:, d, h * S_LOC : (h + 1) * S_LOC],
                        in_=khl_full[2 * h + 1, d * P : (d + 1) * P, :],
                    )
            for j in range(NT):
                nc.sync.dma_start(
                    out=vh[:, j, :], in_=vh_full[j * P : (j + 1) * P, :]
                )

            # ---- attention, one 128-query tile at a time ----
            for st in range(ST):
                scs = [
                    pacc.tile([P, 512], F32, tag="acc", name=f"sc_{st}_{i}")
                    for i in range(4)
                ]
                for t4 in range(4):
                    for p_i, (qa, ka) in enumerate(
                        ((qht_h, kht_h), (qht_h, kht_l), (qht_l, kht_h))
                    ):
                        for d in range(4):
                            nc.tensor.matmul(
                                scs[t4][:],
                                qa[:, d, st * P : (st + 1) * P],
                                ka[:, d, t4 * 512 : (t4 + 1) * 512],
                                start=(p_i == 0 and d == 0),
                                stop=(p_i == 2 and d == 3),
                            )
                s_sb = attn.tile([P, S], F32, tag="ssb")
                for t4 in range(4):
                    nc.vector.scalar_tensor_tensor(
                        out=s_sb[:, t4 * 512 : (t4 + 1) * 512],
                        in0=scs[t4][:],
                        scalar=SCALE,
                        in1=maskb[:, t4 * 512 : (t4 + 1) * 512],
                        op0=mybir.AluOpType.mult,
                        op1=mybir.AluOpType.add,
                    )
                cmax = small.tile([P, 4], F32, tag="cmax")
                for t4 in range(4):
                    nc.vector.tensor_reduce(
                        cmax[:, t4 : t4 + 1], s_sb[:, t4 * 512 : (t4 + 1) * 512],
                        axis=mybir.AxisListType.X, op=mybir.AluOpType.max,
                    )
                nmax = small.tile([P, 1], F32, tag="nmax")
                nc.vector.tensor_reduce(
                    nmax[:], cmax[:],
                    axis=mybir.AxisListType.X, op=mybir.AluOpType.max, negate=True,
                )
                p_sb = attn.tile([P, S], BF16, tag="psb")
                rs4 = small.tile([P, 4], F32, tag="rs4")
                for t4 in range(4):
                    nc.scalar.activation(
                        p_sb[:, t4 * 512 : (t4 + 1) * 512],
                        s_sb[:, t4 * 512 : (t4 + 1) * 512],
                        mybir.ActivationFunctionType.Exp,
                        bias=nmax[:], scale=1.0,
                        accum_out=rs4[:, t4 : t4 + 1],
                    )
                rsum = small.tile([P, 1], F32, tag="rsum")
                nc.vector.tensor_reduce(
                    rsum[:], rs4[:], axis=mybir.AxisListType.X, op=mybir.AluOpType.add,
                )
                rec = small.tile([P, 1], F32, tag="rec")
                nc.vector.reciprocal(rec[:], rsum[:])

                pt_sb = attn.tile([P, S], BF16, tag="ptsb")
                for j in range(NT):
                    pt = ptst.tile([P, P], BF16, tag="tst", name=f"pt_{st}_{j}")
                    nc.tensor.matmul(
                        pt[:], p_sb[:, j * P : (j + 1) * P], identb[:],
                        is_transpose=True,
                    )
                    nc.any.tensor_copy(pt_sb[:, j * P : (j + 1) * P], pt[:])

                po = ppv.tile([P, D], F32, tag="pv")
                for j in range(NT):
                    nc.tensor.matmul(
                        po[:],
                        pt_sb[:, j * P : (j + 1) * P],
                        vh[:, j, :],
                        start=(j == 0),
                        stop=(j == NT - 1),
                    )
                osb = io.tile([P, D], F32, tag="osb", bufs=2)
                nc.scalar.mul(osb[:], po[:], mul=rec[:])
                nc.sync.dma_start(out=out[st * P : (st + 1) * P, :], in_=osb[:])

    nc.compile()
    return nc


def _get_compiled():
    global _COMPILED
    if _COMPILED is None:
        _COMPILED = _build()
    return _COMPILED


def _split16_packed(a):
    hi = a.astype(np.float16)
    lo = (a - hi.astype(np.float32)).astype(np.float16)
    return np.ascontiguousarray(np.stack([hi, lo], axis=1))


def kernel(q, k, v, mask, Wq, Wk, Wv, **_unused):
    import ml_dtypes

    q = np.asarray(q, dtype=np.float32)
    k = np.asarray(k, dtype=np.float32)
    v = np.asarray(v, dtype=np.float32)
    mask = np.asarray(mask)
    wqt = _split16_packed(np.ascontiguousarray(np.asarray(Wq, dtype=np.float32).T))
    wkt = _split16_packed(np.ascontiguousarray(np.asarray(Wk, dtype=np.float32).T))
    wvt = np.ascontiguousarray(
        np.asarray(Wv, dtype=np.float32).T.astype(ml_dtypes.bfloat16)
    )
    maskf = np.where(mask == 0, np.float32(NEG), np.float32(0.0)).astype(np.float32)

    nc = _get_compiled()

    in_maps = []
    for c in range(N_CORES):
        b, h = divmod(c, 2)
        rows = slice(h * S_LOC, (h + 1) * S_LOC)
        in_maps.append(
            {
                "q": np.ascontiguousarray(q[b, rows]),
                "k": np.ascontiguousarray(k[b, rows]),
                "v": np.ascontiguousarray(v[b, rows]),
                "wqt": wqt,
                "wkt": wkt,
                "wvt": wvt,
                "maskf": maskf[b : b + 1],
            }
        )

    trace = bool(int(os.environ.get("KERNEL_TRACE", "0")))
    res = bass_utils.run_bass_kernel_spmd(
        nc, in_maps, core_ids=list(range(N_CORES)), trace=trace
    )
    if trace:
        kernel.last_exec_time_ns = res.exec_time_ns

    full = np.empty((B, S, D), dtype=np.float32)
    for c in range(N_CORES):
        b, h = divmod(c, 2)
        full[b, h * S_LOC : (h + 1) * S_LOC] = res.results[c]["out"]
    return full


kernel.last_exec_time_ns = None


# revision 32
# speedup vs baseline: 1.0058x; 1.0058x over previous
"""Distributed single-head attention on 8 TRN2 NeuronCores.

Reference computation (fp32):
    qh = q @ Wq.T ; kh = k @ Wk.T ; vh = v @ Wv.T          [B,S,512]
    scores = (qh @ kh.T) * sqrt(4096)                       [B,S,S]
    scores = where(mask==0, -1e9, scores)
    out = softmax(scores, -1) @ vh                          [B,S,512]
with B=4, S=2048, HIDDEN=4096, HEAD=512.

Sharding: 8 cores = (batch b, seq half h); core c handles query rows
[h*1024, (h+1)*1024) of batch b = c//2.  Each core projects only its own
1024 rows of q/k/v; the pair of cores sharing a batch exchanges khT / vh
via an intra-pair AllGather, overlapped with the q projection.

Precision: the softmax is saturated (score std ~1450 after the *64
scale) so the output is ~one-hot @ argmax of the masked scores — score
precision decides correctness.  The q/k projections and QK^T run as
3-pass fp16 hi/lo split matmuls (hi@hi + hi@lo + lo@hi, fp32 PSUM
accumulation): measured 2.2e-7 relative error on silicon — full fp32
quality at 3 PE cycles/row instead of fp32's 4.  The V path only feeds
the post-softmax average, so it runs in bf16 (1 cycle/row, ~3e-3
relative output error).  PE transposes are bit-exact (verified) and put
the contraction dim on partitions for q/k/v and for P^T in PV.
"""

import os
import sys

import numpy as np


def _ensure_path():
    for p in ("/opt/trn_rl_repo", "/opt/pypackages"):
        if os.path.isdir(p) and p not in sys.path:
            sys.path.append(p)


_ensure_path()

from concourse import bacc, masks, tile  # noqa: E402
from concourse import bass_utils  # noqa: E402
from concourse.bass import mybir  # noqa: E402

# S3 upload is unavailable in this container; keep profile artifacts local.
bass_utils.upload_artifacts = lambda tmpdir: tmpdir

F32 = mybir.dt.float32
F16 = mybir.dt.float16
BF16 = mybir.dt.bfloat16

B, S, E, D = 4, 2048, 4096, 512
N_CORES = 8
S_LOC = B * S // N_CORES  # 1024 rows per core
SCALE = float(E) ** 0.5  # 64.0
NEG = -1e9

P = 128
EC = E // P  # 32 contraction chunks for projections
DC = D // P  # 4 head-dim chunks
NT = S // P  # 16 key tiles
ST = S_LOC // P  # 8 query tiles per core

REPLICA_GROUPS = [[0, 1], [2, 3], [4, 5], [6, 7]]

_COMPILED = None


def _build():
    nc = bacc.Bacc("TRN2", target_bir_lowering=False, debug=False, num_devices=N_CORES)

    q_in = nc.dram_tensor("q", [S_LOC, E], F32, kind="ExternalInput").ap()
    k_in = nc.dram_tensor("k", [S_LOC, E], F32, kind="ExternalInput").ap()
    v_in = nc.dram_tensor("v", [S_LOC, E], F32, kind="ExternalInput").ap()
    # W.T hi/lo fp16 pairs packed as [E, 2, D] (host-prepacked)
    wqt = nc.dram_tensor("wqt", [E, 2, D], F16, kind="ExternalInput").ap()
    wkt = nc.dram_tensor("wkt", [E, 2, D], F16, kind="ExternalInput").ap()
    wvt = nc.dram_tensor("wvt", [E, D], BF16, kind="ExternalInput").ap()
    maskf = nc.dram_tensor("maskf", [1, S], F32, kind="ExternalInput").ap()
    out = nc.dram_tensor("out", [S_LOC, D], F32, kind="ExternalOutput").ap()

    # Internal DRAM bounce buffers for the intra-pair AllGathers.
    # khl_loc packs khT hi at [0] and lo at [1] (fp16).
    khl_loc = nc.dram_tensor("khl_loc", [2, D, S_LOC], F16).ap()
    khl_full = nc.dram_tensor("khl_full", [4, D, S_LOC], F16).ap()
    vh_loc = nc.dram_tensor("vh_loc", [S_LOC, D], BF16).ap()
    vh_full = nc.dram_tensor("vh_full", [S, D], BF16).ap()

    with tile.TileContext(nc) as tc:
        with (
            tc.tile_pool(name="const", bufs=1) as const,
            tc.tile_pool(name="big", bufs=1) as big,
            tc.tile_pool(name="io", bufs=3) as io,
            tc.tile_pool(name="attn", bufs=2) as attn,
            tc.tile_pool(name="small", bufs=4) as small,
            tc.tile_pool(name="pacc", bufs=5, space="PSUM") as pacc,
            tc.tile_pool(name="ptst", bufs=2, space="PSUM") as ptst,
            tc.tile_pool(name="ppv", bufs=1, space="PSUM") as ppv,
        ):
            # ---- constants ----
            ident = const.tile([P, P], F32, tag="ident")
            masks.make_identity(nc, ident[:])
            identb = const.tile([P, P], BF16, tag="identb")
            masks.make_identity(nc, identb[:])
            # maskb[p, t] = maskf[t] for all partitions (0-stride broadcast DMA).
            maskb = const.tile([P, S], F32, tag="maskb")
            nc.sync.dma_start(out=maskb[:], in_=maskf[:].to_broadcast((P, S)))

            # persistent per-core tensors (fp16 hi/lo for the score path)
            qht_h = big.tile([P, DC, S_LOC], F16, tag="qht_h")
            qht_l = big.tile([P, DC, S_LOC], F16, tag="qht_l")
            kht_h = big.tile([P, DC, S], F16, tag="kht_h")
            kht_l = big.tile([P, DC, S], F16, tag="kht_l")
            vh = big.tile([P, NT, D], BF16, tag="vh")

            def load_sup(x_in, g, e4, tag, bufs=3):
                """Load a [512 rows x 512 cols] block of x as [128, 4, 512]
                (8 contiguous-row DMAs on SWDGE, 1KB runs)."""
                xs = io.tile([P, 4, 512], F32, tag=tag, name=f"{tag}_{g}_{e4}", bufs=bufs)
                for j in range(4):
                    for hh in range(2):
                        nc.gpsimd.dma_start(
                            out=xs[:, j, hh * 256 : (hh + 1) * 256],
                            in_=x_in[
                                g * 512 + j * P : g * 512 + (j + 1) * P,
                                e4 * 512 + hh * 256 : e4 * 512 + (hh + 1) * 256,
                            ],
                        )
                return xs

            def project_hl(x_in, w_in, sink, xtag, xbufs=3):
                """fp16 hi/lo projection: psum tiles [d 128, s 512] accumulated
                over 32 e-chunks x 3 passes.  One-iteration software pipeline:
                transposes/casts for e run while matmuls for e-1 execute."""
                for g in range(2):
                    accs = [
                        pacc.tile([P, 512], F32, tag="acc", name=f"acc_{g}_{i}")
                        for i in range(4)
                    ]

                    def stage(e):
                        if e % 4 == 0:
                            stage.xs = load_sup(x_in, g, e // 4, xtag, bufs=xbufs)
                        whl = io.tile(
                            [P, 2, D], F16, tag="whl", name=f"whl_{g}_{e}", bufs=3
                        )
                        nc.scalar.dma_start(
                            out=whl[:], in_=w_in[e * P : (e + 1) * P, :, :]
                        )
                        el = (e % 4) * P
                        trh = io.tile([P, 512], F16, tag="xTh")
                        trl = io.tile([P, 512], F16, tag="xTl")
                        for j in range(4):
                            pt = ptst.tile([P, P], F32, tag="tst", name=f"t_{g}_{e}_{j}")
                            nc.tensor.matmul(
                                pt[:], stage.xs[:, j, el : el + P], ident[:],
                                is_transpose=True,
                            )
                            nc.any.tensor_copy(trh[:, j * P : (j + 1) * P], pt[:])
                            nc.vector.scalar_tensor_tensor(
                                out=trl[:, j * P : (j + 1) * P],
                                in0=trh[:, j * P : (j + 1) * P], scalar=-1.0,
                                in1=pt[:],
                                op0=mybir.AluOpType.mult, op1=mybir.AluOpType.add,
                            )
                        return whl, trh, trl

                    def matmuls(e, whl, trh, trl):
                        for d in range(4):
                            for p_i, (w_i, xt16) in enumerate(
                                ((0, trh), (0, trl), (1, trh))
                            ):
                                nc.tensor.matmul(
                                    accs[d][:],
                                    whl[:, w_i, d * P : (d + 1) * P],
                                    xt16[:],
                                    start=(e == 0 and p_i == 0),
                                    stop=(e == EC - 1 and p_i == 2),
                                )

                    prev = None
                    for e in range(EC):
                        cur = stage(e)
                        if prev is not None:
                            matmuls(e - 1, *prev)
                        prev = cur
                    matmuls(EC - 1, *prev)

                    for j in range(4):
                        sink(g, j, accs[j][:])

            def split_hl(ps, hi_ap, lo_ap):
                """Evict fp32 psum into fp16 hi + lo."""
                nc.any.tensor_copy(hi_ap, ps)
                nc.vector.scalar_tensor_tensor(
                    out=lo_ap, in0=hi_ap, scalar=-1.0, in1=ps,
                    op0=mybir.AluOpType.mult, op1=mybir.AluOpType.add,
                )

            # ---- v projection (bf16, software-pipelined) -> DRAM bounce ----
            def v_sink(g, j, ps):
                stg = io.tile([P, 512], BF16, tag="postgb", name=f"vs_{g}_{j}", bufs=2)
                nc.any.tensor_copy(stg[:], ps)
                t0 = (g * 4 + j) * P
                nc.sync.dma_start(out=vh_loc[t0 : t0 + P, :], in_=stg[:])

            for g in range(2):
                vaccs = [
                    pacc.tile([P, 512], F32, tag="acc", name=f"vacc_{g}_{i}")
                    for i in range(4)
                ]

                def vstage(e, g=g):
                    if e % 4 == 0:
                        vstage.xs = load_sup(v_in, g, e // 4, "xsupv")
                        vstage.xb = io.tile(
                            [P, 4, 512], BF16, tag="xbsup",
                            name=f"xb_{g}_{e}", bufs=2,
                        )
                        nc.scalar.copy(vstage.xb[:], vstage.xs[:])
                    w = io.tile([P, D], BF16, tag="wloadb", name=f"wv_{g}_{e}", bufs=6)
                    nc.scalar.dma_start(out=w[:], in_=wvt[e * P : (e + 1) * P, :])
                    el = (e % 4) * P
                    trjs = []
                    for j in range(4):
                        pt = ptst.tile([P, P], BF16, tag="tst", name=f"vt_{g}_{e}_{j}")
                        nc.tensor.matmul(
                            pt[:], vstage.xb[:, j, el : el + P], identb[:],
                            is_transpose=True,
                        )
                        trj = io.tile([P, P], BF16, tag="xTj", name=f"vj_{g}_{e}_{j}", bufs=6)
                        nc.any.tensor_copy(trj[:], pt[:])
                        trjs.append(trj)
                    return w, trjs

                def vmatmuls(e, w, trjs, vaccs=vaccs):
                    for j in range(4):
                        nc.tensor.matmul(
                            vaccs[j][:], trjs[j][:], w[:],
                            start=(e == 0), stop=(e == EC - 1),
                        )

                prev = None
                for e in range(EC):
                    cur = vstage(e)
                    if prev is not None:
                        vmatmuls(e - 1, *prev)
                    prev = cur
                vmatmuls(EC - 1, *prev)
                for j in range(4):
                    v_sink(g, j, vaccs[j][:])

            # vh AllGather fires as soon as vh_loc is written (overlaps k+q).
            nc.gpsimd.collective_compute(
                "AllGather",
                mybir.AluOpType.bypass,
                replica_groups=REPLICA_GROUPS,
                ins=[vh_loc.opt()],
                outs=[vh_full.opt()],
            )

            # ---- k projection -> khT hi/lo -> DRAM bounce ----
            def k_sink(g, d, ps):
                sh = io.tile([P, 512], F16, tag="postgh", name=f"ksh_{g}_{d}", bufs=2)
                sl = io.tile([P, 512], F16, tag="postgl", name=f"ksl_{g}_{d}", bufs=2)
                split_hl(ps, sh[:], sl[:])
                nc.sync.dma_start(
                    out=khl_loc[0, d * P : (d + 1) * P, g * 512 : (g + 1) * 512],
                    in_=sh[:],
                )
                nc.sync.dma_start(
                    out=khl_loc[1, d * P : (d + 1) * P, g * 512 : (g + 1) * 512],
                    in_=sl[:],
                )

            project_hl(k_in, wkt, k_sink, "xsupk")

            nc.gpsimd.collective_compute(
                "AllGather",
                mybir.AluOpType.bypass,
                replica_groups=REPLICA_GROUPS,
                ins=[khl_loc.opt()],
                outs=[khl_full.opt()],
            )

            # ---- q projection -> qht hi/lo (stays in SBUF) ----
            def q_sink(g, d, ps):
                split_hl(
                    ps,
                    qht_h[:, d, g * 512 : (g + 1) * 512],
                    qht_l[:, d, g * 512 : (g + 1) * 512],
                )

            project_hl(q_in, wqt, q_sink, "xsupq", xbufs=2)

            # ---- gather AG results back to SBUF (split DMAs across engines) ----
            # khl_full[h*2 + {0,1}] = rank-h's khT {hi,lo}.
            for h in range(2):
                for d in range(DC):
                    nc.sync.dma_start(
                        out=kht_h[:, d, h * S_LOC : (h + 1) * S_LOC],
                        in_=khl_full[2 * h, d * P : (d + 1) * P, :],
                    )
                    nc.sync.dma_start(
                        out=kht_l[:, d, h * S_LOC : (h + 1) * S_LOC],
                        in_=khl_full[2 * h + 1, d * P : (d + 1) * P, :],
                    )
            for j in range(NT):
                nc.sync.dma_start(
                    out=vh[:, j, :], in_=vh_full[j * P : (j + 1) * P, :]
                )

            # ---- attention, one 128-query tile at a time ----
            for st in range(ST):
                scs = [
                    pacc.tile([P, 512], F32, tag="acc", name=f"sc_{st}_{i}")
                    for i in range(4)
                ]
                for t4 in range(4):
                    for p_i, (qa, ka) in enumerate(
                        ((qht_h, kht_h), (qht_h, kht_l), (qht_l, kht_h))
                    ):
                        for d in range(4):
                            nc.tensor.matmul(
                                scs[t4][:],
                                qa[:, d, st * P : (st + 1) * P],
                                ka[:, d, t4 * 512 : (t4 + 1) * 512],
                                start=(p_i == 0 and d == 0),
                                stop=(p_i == 2 and d == 3),
                            )
                s_sb = attn.tile([P, S], F32, tag="ssb")
                for t4 in range(4):
                    nc.vector.scalar_tensor_tensor(
                        out=s_sb[:, t4 * 512 : (t4 + 1) * 512],
                        in0=scs[t4][:],
                        scalar=SCALE,
                        in1=maskb[:, t4 * 512 : (t4 + 1) * 512],
                        op0=mybir.AluOpType.mult,
                        op1=mybir.AluOpType.add,
                    )
                cmax = small.tile([P, 4], F32, tag="cmax")
                for t4 in range(4):
                    nc.vector.tensor_reduce(
                        cmax[:, t4 : t4 + 1], s_sb[:, t4 * 512 : (t4 + 1) * 512],
                        axis=mybir.AxisListType.X, op=mybir.AluOpType.max,
                    )
                nmax = small.tile([P, 1], F32, tag="nmax")
                nc.vector.tensor_reduce(
                    nmax[:], cmax[:],
                    axis=mybir.AxisListType.X, op=mybir.AluOpType.max, negate=True,
                )
                p_sb = attn.tile([P, S], BF16, tag="psb")
                rs4 = small.tile([P, 4], F32, tag="rs4")
                for t4 in range(4):
                    nc.scalar.activation(
                        p_sb[:, t4 * 512 : (t4 + 1) * 512],
                        s_sb[:, t4 * 512 : (t4 + 1) * 512],
                        mybir.ActivationFunctionType.Exp,
                        bias=nmax[:], scale=1.0,
                        accum_out=rs4[:, t4 : t4 + 1],
                    )
                rsum = small.tile([P, 1], F32, tag="rsum")
                nc.vector.tensor_reduce(
                    rsum[:], rs4[:], axis=mybir.AxisListType.X, op=mybir.AluOpType.add,
                )
                rec = small.tile([P, 1], F32, tag="rec")
                nc.vector.reciprocal(rec[:], rsum[:])

                pt_sb = attn.tile([P, S], BF16, tag="ptsb")
                for j in range(NT):
                    pt = ptst.tile([P, P], BF16, tag="tst", name=f"pt_{st}_{j}")
                    nc.tensor.matmul(
                        pt[:], p_sb[:, j * P : (j + 1) * P], identb[:],
                        is_transpose=True,
                    )
                    nc.any.tensor_copy(pt_sb[:, j * P : (j + 1) * P], pt[:])

                po = ppv.tile([P, D], F32, tag="pv")
                for j in range(NT):
                    nc.tensor.matmul(
                        po[:],
                        pt_sb[:, j * P : (j + 1) * P],
                        vh[:, j, :],
                        start=(j == 0),
                        stop=(j == NT - 1),
                    )
                osb = io.tile([P, D], F32, tag="osb", bufs=2)
                nc.scalar.mul(osb[:], po[:], mul=rec[:])
                nc.sync.dma_start(out=out[st * P : (st + 1) * P, :], in_=osb[:])

    nc.compile()
    return nc


def _get_compiled():
    global _COMPILED
    if _COMPILED is None:
        _COMPILED = _build()
    return _COMPILED


def _split16_packed(a):
    hi = a.astype(np.float16)
    lo = (a - hi.astype(np.float32)).astype(np.float16)
    return np.ascontiguousarray(np.stack([hi, lo], axis=1))


def kernel(q, k, v, mask, Wq, Wk, Wv, **_unused):
    import ml_dtypes

    q = np.asarray(q, dtype=np.float32)
    k = np.asarray(k, dtype=np.float32)
    v = np.asarray(v, dtype=np.float32)
    mask = np.asarray(mask)
    wqt = _split16_packed(np.ascontiguousarray(np.asarray(Wq, dtype=np.float32).T))
    wkt = _split16_packed(np.ascontiguousarray(np.asarray(Wk, dtype=np.float32).T))
    wvt = np.ascontiguousarray(
        np.asarray(Wv, dtype=np.float32).T.astype(ml_dtypes.bfloat16)
    )
    maskf = np.where(mask == 0, np.float32(NEG), np.float32(0.0)).astype(np.float32)

    nc = _get_compiled()

    in_maps = []
    for c in range(N_CORES):
        b, h = divmod(c, 2)
        rows = slice(h * S_LOC, (h + 1) * S_LOC)
        in_maps.append(
            {
                "q": np.ascontiguousarray(q[b, rows]),
                "k": np.ascontiguousarray(k[b, rows]),
                "v": np.ascontiguousarray(v[b, rows]),
                "wqt": wqt,
                "wkt": wkt,
                "wvt": wvt,
                "maskf": maskf[b : b + 1],
            }
        )

    trace = bool(int(os.environ.get("KERNEL_TRACE", "0")))
    res = bass_utils.run_bass_kernel_spmd(
        nc, in_maps, core_ids=list(range(N_CORES)), trace=trace
    )
    if trace:
        kernel.last_exec_time_ns = res.exec_time_ns

    full = np.empty((B, S, D), dtype=np.float32)
    for c in range(N_CORES):
        b, h = divmod(c, 2)
        full[b, h * S_LOC : (h + 1) * S_LOC] = res.results[c]["out"]
    return full


kernel.last_exec_time_ns = None


# revision 33
# speedup vs baseline: 1.0104x; 1.0046x over previous
"""Distributed single-head attention on 8 TRN2 NeuronCores.

Reference computation (fp32):
    qh = q @ Wq.T ; kh = k @ Wk.T ; vh = v @ Wv.T          [B,S,512]
    scores = (qh @ kh.T) * sqrt(4096)                       [B,S,S]
    scores = where(mask==0, -1e9, scores)
    out = softmax(scores, -1) @ vh                          [B,S,512]
with B=4, S=2048, HIDDEN=4096, HEAD=512.

Sharding: 8 cores = (batch b, seq half h); core c handles query rows
[h*1024, (h+1)*1024) of batch b = c//2.  Each core projects only its own
1024 rows of q/k/v; the pair of cores sharing a batch exchanges khT / vh
via an intra-pair AllGather, overlapped with the q projection.

Precision: the softmax is saturated (score std ~1450 after the *64
scale) so the output is ~one-hot @ argmax of the masked scores — score
precision decides correctness.  The q/k projections and QK^T run as
3-pass fp16 hi/lo split matmuls (hi@hi + hi@lo + lo@hi, fp32 PSUM
accumulation): measured 2.2e-7 relative error on silicon — full fp32
quality at 3 PE cycles/row instead of fp32's 4.  The V path only feeds
the post-softmax average, so it runs in bf16 (1 cycle/row, ~3e-3
relative output error).  PE transposes are bit-exact (verified) and put
the contraction dim on partitions for q/k/v and for P^T in PV.
"""

import os
import sys

import numpy as np


def _ensure_path():
    for p in ("/opt/trn_rl_repo", "/opt/pypackages"):
        if os.path.isdir(p) and p not in sys.path:
            sys.path.append(p)


_ensure_path()

from concourse import bacc, masks, tile  # noqa: E402
from concourse import bass_utils  # noqa: E402
from concourse.bass import mybir  # noqa: E402

# S3 upload is unavailable in this container; keep profile artifacts local.
bass_utils.upload_artifacts = lambda tmpdir: tmpdir

F32 = mybir.dt.float32
F16 = mybir.dt.float16
BF16 = mybir.dt.bfloat16

B, S, E, D = 4, 2048, 4096, 512
N_CORES = 8
S_LOC = B * S // N_CORES  # 1024 rows per core
SCALE = float(E) ** 0.5  # 64.0
NEG = -1e9

P = 128
EC = E // P  # 32 contraction chunks for projections
DC = D // P  # 4 head-dim chunks
NT = S // P  # 16 key tiles
ST = S_LOC // P  # 8 query tiles per core

REPLICA_GROUPS = [[0, 1], [2, 3], [4, 5], [6, 7]]

_COMPILED = None


def _build():
    nc = bacc.Bacc("TRN2", target_bir_lowering=False, debug=False, num_devices=N_CORES)

    q_in = nc.dram_tensor("q", [S_LOC, E], F32, kind="ExternalInput").ap()
    k_in = nc.dram_tensor("k", [S_LOC, E], F32, kind="ExternalInput").ap()
    v_in = nc.dram_tensor("v", [S_LOC, E], F32, kind="ExternalInput").ap()
    # W.T hi/lo fp16 pairs packed as [E, 2, D] (host-prepacked)
    wqt = nc.dram_tensor("wqt", [E, 2, D], F16, kind="ExternalInput").ap()
    wkt = nc.dram_tensor("wkt", [E, 2, D], F16, kind="ExternalInput").ap()
    wvt = nc.dram_tensor("wvt", [E, D], BF16, kind="ExternalInput").ap()
    maskf = nc.dram_tensor("maskf", [1, S], BF16, kind="ExternalInput").ap()
    out = nc.dram_tensor("out", [S_LOC, D], F32, kind="ExternalOutput").ap()

    # Internal DRAM bounce buffers for the intra-pair AllGathers.
    # khl_loc packs khT hi at [0] and lo at [1] (fp16).
    khl_loc = nc.dram_tensor("khl_loc", [2, D, S_LOC], F16).ap()
    khl_full = nc.dram_tensor("khl_full", [4, D, S_LOC], F16).ap()
    vh_loc = nc.dram_tensor("vh_loc", [S_LOC, D], BF16).ap()
    vh_full = nc.dram_tensor("vh_full", [S, D], BF16).ap()

    with tile.TileContext(nc) as tc:
        with (
            tc.tile_pool(name="const", bufs=1) as const,
            tc.tile_pool(name="big", bufs=1) as big,
            tc.tile_pool(name="io", bufs=3) as io,
            tc.tile_pool(name="attn", bufs=2) as attn,
            tc.tile_pool(name="small", bufs=4) as small,
            tc.tile_pool(name="pacc", bufs=5, space="PSUM") as pacc,
            tc.tile_pool(name="ptst", bufs=2, space="PSUM") as ptst,
            tc.tile_pool(name="ppv", bufs=1, space="PSUM") as ppv,
        ):
            # ---- constants ----
            ident = const.tile([P, P], F32, tag="ident")
            masks.make_identity(nc, ident[:])
            identb = const.tile([P, P], BF16, tag="identb")
            masks.make_identity(nc, identb[:])
            # maskb[p, t] = maskf[t] for all partitions (0-stride broadcast DMA).
            # bf16: holds only 0.0 / ~-1e9; masked scores underflow to 0 in exp
            # either way, unmasked get exactly +0.0 — no precision impact.
            maskb = const.tile([P, S], BF16, tag="maskb")
            nc.sync.dma_start(out=maskb[:], in_=maskf[:].to_broadcast((P, S)))

            # persistent per-core tensors (fp16 hi/lo for the score path)
            qht_h = big.tile([P, DC, S_LOC], F16, tag="qht_h")
            qht_l = big.tile([P, DC, S_LOC], F16, tag="qht_l")
            kht_h = big.tile([P, DC, S], F16, tag="kht_h")
            kht_l = big.tile([P, DC, S], F16, tag="kht_l")
            vh = big.tile([P, NT, D], BF16, tag="vh")

            def load_sup(x_in, g, e4, tag, bufs=3):
                """Load a [512 rows x 512 cols] block of x as [128, 4, 512]
                (8 contiguous-row DMAs on SWDGE, 1KB runs)."""
                xs = io.tile([P, 4, 512], F32, tag=tag, name=f"{tag}_{g}_{e4}", bufs=bufs)
                for j in range(4):
                    for hh in range(2):
                        nc.gpsimd.dma_start(
                            out=xs[:, j, hh * 256 : (hh + 1) * 256],
                            in_=x_in[
                                g * 512 + j * P : g * 512 + (j + 1) * P,
                                e4 * 512 + hh * 256 : e4 * 512 + (hh + 1) * 256,
                            ],
                        )
                return xs

            def project_hl(x_in, w_in, sink, xtag, xbufs=3):
                """fp16 hi/lo projection: psum tiles [d 128, s 512] accumulated
                over 32 e-chunks x 3 passes.  One-iteration software pipeline:
                transposes/casts for e run while matmuls for e-1 execute."""
                for g in range(2):
                    accs = [
                        pacc.tile([P, 512], F32, tag="acc", name=f"acc_{g}_{i}")
                        for i in range(4)
                    ]

                    def stage(e):
                        if e % 4 == 0:
                            stage.xs = load_sup(x_in, g, e // 4, xtag, bufs=xbufs)
                        whl = io.tile(
                            [P, 2, D], F16, tag="whl", name=f"whl_{g}_{e}", bufs=5
                        )
                        nc.scalar.dma_start(
                            out=whl[:], in_=w_in[e * P : (e + 1) * P, :, :]
                        )
                        el = (e % 4) * P
                        trh = io.tile([P, 512], F16, tag="xTh")
                        trl = io.tile([P, 512], F16, tag="xTl")
                        for j in range(4):
                            pt = ptst.tile([P, P], F32, tag="tst", name=f"t_{g}_{e}_{j}")
                            nc.tensor.matmul(
                                pt[:], stage.xs[:, j, el : el + P], ident[:],
                                is_transpose=True,
                            )
                            nc.any.tensor_copy(trh[:, j * P : (j + 1) * P], pt[:])
                            nc.vector.scalar_tensor_tensor(
                                out=trl[:, j * P : (j + 1) * P],
                                in0=trh[:, j * P : (j + 1) * P], scalar=-1.0,
                                in1=pt[:],
                                op0=mybir.AluOpType.mult, op1=mybir.AluOpType.add,
                            )
                        return whl, trh, trl

                    def matmuls(e, whl, trh, trl):
                        for d in range(4):
                            for p_i, (w_i, xt16) in enumerate(
                                ((0, trh), (0, trl), (1, trh))
                            ):
                                nc.tensor.matmul(
                                    accs[d][:],
                                    whl[:, w_i, d * P : (d + 1) * P],
                                    xt16[:],
                                    start=(e == 0 and p_i == 0),
                                    stop=(e == EC - 1 and p_i == 2),
                                )

                    prev = None
                    for e in range(EC):
                        cur = stage(e)
                        if prev is not None:
                            matmuls(e - 1, *prev)
                        prev = cur
                    matmuls(EC - 1, *prev)

                    for j in range(4):
                        sink(g, j, accs[j][:])

            def split_hl(ps, hi_ap, lo_ap):
                """Evict fp32 psum into fp16 hi + lo."""
                nc.any.tensor_copy(hi_ap, ps)
                nc.vector.scalar_tensor_tensor(
                    out=lo_ap, in0=hi_ap, scalar=-1.0, in1=ps,
                    op0=mybir.AluOpType.mult, op1=mybir.AluOpType.add,
                )

            # ---- v projection (bf16, software-pipelined) -> DRAM bounce ----
            def v_sink(g, j, ps):
                stg = io.tile([P, 512], BF16, tag="postgb", name=f"vs_{g}_{j}", bufs=2)
                nc.any.tensor_copy(stg[:], ps)
                t0 = (g * 4 + j) * P
                nc.sync.dma_start(out=vh_loc[t0 : t0 + P, :], in_=stg[:])

            for g in range(2):
                vaccs = [
                    pacc.tile([P, 512], F32, tag="acc", name=f"vacc_{g}_{i}")
                    for i in range(4)
                ]

                def vstage(e, g=g):
                    if e % 4 == 0:
                        vstage.xs = load_sup(v_in, g, e // 4, "xsupv")
                        vstage.xb = io.tile(
                            [P, 4, 512], BF16, tag="xbsup",
                            name=f"xb_{g}_{e}", bufs=2,
                        )
                        nc.scalar.copy(vstage.xb[:], vstage.xs[:])
                    w = io.tile([P, D], BF16, tag="wloadb", name=f"wv_{g}_{e}", bufs=6)
                    nc.scalar.dma_start(out=w[:], in_=wvt[e * P : (e + 1) * P, :])
                    el = (e % 4) * P
                    trjs = []
                    for j in range(4):
                        pt = ptst.tile([P, P], BF16, tag="tst", name=f"vt_{g}_{e}_{j}")
                        nc.tensor.matmul(
                            pt[:], vstage.xb[:, j, el : el + P], identb[:],
                            is_transpose=True,
                        )
                        trj = io.tile([P, P], BF16, tag="xTj", name=f"vj_{g}_{e}_{j}", bufs=6)
                        nc.any.tensor_copy(trj[:], pt[:])
                        trjs.append(trj)
                    return w, trjs

                def vmatmuls(e, w, trjs, vaccs=vaccs):
                    for j in range(4):
                        nc.tensor.matmul(
                            vaccs[j][:], trjs[j][:], w[:],
                            start=(e == 0), stop=(e == EC - 1),
                        )

                prev = None
                for e in range(EC):
                    cur = vstage(e)
                    if prev is not None:
                        vmatmuls(e - 1, *prev)
                    prev = cur
                vmatmuls(EC - 1, *prev)
                for j in range(4):
                    v_sink(g, j, vaccs[j][:])

            # vh AllGather fires as soon as vh_loc is written (overlaps k+q).
            nc.gpsimd.collective_compute(
                "AllGather",
                mybir.AluOpType.bypass,
                replica_groups=REPLICA_GROUPS,
                ins=[vh_loc.opt()],
                outs=[vh_full.opt()],
            )

            # ---- k projection -> khT hi/lo -> DRAM bounce ----
            def k_sink(g, d, ps):
                sh = io.tile([P, 512], F16, tag="postgh", name=f"ksh_{g}_{d}", bufs=2)
                sl = io.tile([P, 512], F16, tag="postgl", name=f"ksl_{g}_{d}", bufs=2)
                split_hl(ps, sh[:], sl[:])
                nc.sync.dma_start(
                    out=khl_loc[0, d * P : (d + 1) * P, g * 512 : (g + 1) * 512],
                    in_=sh[:],
                )
                nc.sync.dma_start(
                    out=khl_loc[1, d * P : (d + 1) * P, g * 512 : (g + 1) * 512],
                    in_=sl[:],
                )

            project_hl(k_in, wkt, k_sink, "xsupk")

            nc.gpsimd.collective_compute(
                "AllGather",
                mybir.AluOpType.bypass,
                replica_groups=REPLICA_GROUPS,
                ins=[khl_loc.opt()],
                outs=[khl_full.opt()],
            )

            # ---- q projection -> qht hi/lo (stays in SBUF) ----
            def q_sink(g, d, ps):
                split_hl(
                    ps,
                    qht_h[:, d, g * 512 : (g + 1) * 512],
                    qht_l[:, d, g * 512 : (g + 1) * 512],
                )

            project_hl(q_in, wqt, q_sink, "xsupq", xbufs=2)

            # ---- gather AG results back to SBUF (split DMAs across engines) ----
            # khl_full[h*2 + {0,1}] = rank-h's khT {hi,lo}.
            for h in range(2):
                for d in range(DC):
                    nc.sync.dma_start(
                        out=kht_h[:, d, h * S_LOC : (h + 1) * S_LOC],
                        in_=khl_full[2 * h, d * P : (d + 1) * P, :],
                    )
                    nc.sync.dma_start(
                        out=kht_l[:, d, h * S_LOC : (h + 1) * S_LOC],
                        in_=khl_full[2 * h + 1, d * P : (d + 1) * P, :],
                    )
            for j in range(NT):
                nc.sync.dma_start(
                    out=vh[:, j, :], in_=vh_full[j * P : (j + 1) * P, :]
                )

            # ---- attention, one 128-query tile at a time ----
            for st in range(ST):
                scs = [
                    pacc.tile([P, 512], F32, tag="acc", name=f"sc_{st}_{i}")
                    for i in range(4)
                ]
                for t4 in range(4):
                    for p_i, (qa, ka) in enumerate(
                        ((qht_h, kht_h), (qht_h, kht_l), (qht_l, kht_h))
                    ):
                        for d in range(4):
                            nc.tensor.matmul(
                                scs[t4][:],
                                qa[:, d, st * P : (st + 1) * P],
                                ka[:, d, t4 * 512 : (t4 + 1) * 512],
                                start=(p_i == 0 and d == 0),
                                stop=(p_i == 2 and d == 3),
                            )
                s_sb = attn.tile([P, S], F32, tag="ssb")
                for t4 in range(4):
                    nc.vector.scalar_tensor_tensor(
                        out=s_sb[:, t4 * 512 : (t4 + 1) * 512],
                        in0=scs[t4][:],
                        scalar=SCALE,
                        in1=maskb[:, t4 * 512 : (t4 + 1) * 512],
                        op0=mybir.AluOpType.mult,
                        op1=mybir.AluOpType.add,
                    )
                cmax = small.tile([P, 4], F32, tag="cmax")
                for t4 in range(4):
                    nc.vector.tensor_reduce(
                        cmax[:, t4 : t4 + 1], s_sb[:, t4 * 512 : (t4 + 1) * 512],
                        axis=mybir.AxisListType.X, op=mybir.AluOpType.max,
                    )
                nmax = small.tile([P, 1], F32, tag="nmax")
                nc.vector.tensor_reduce(
                    nmax[:], cmax[:],
                    axis=mybir.AxisListType.X, op=mybir.AluOpType.max, negate=True,
                )
                p_sb = attn.tile([P, S], BF16, tag="psb")
                rs4 = small.tile([P, 4], F32, tag="rs4")
                for t4 in range(4):
                    nc.scalar.activation(
                        p_sb[:, t4 * 512 : (t4 + 1) * 512],
                        s_sb[:, t4 * 512 : (t4 + 1) * 512],
                        mybir.ActivationFunctionType.Exp,
                        bias=nmax[:], scale=1.0,
                        accum_out=rs4[:, t4 : t4 + 1],
                    )
                rsum = small.tile([P, 1], F32, tag="rsum")
                nc.vector.tensor_reduce(
                    rsum[:], rs4[:], axis=mybir.AxisListType.X, op=mybir.AluOpType.add,
                )
                rec = small.tile([P, 1], F32, tag="rec")
                nc.vector.reciprocal(rec[:], rsum[:])

                pt_sb = attn.tile([P, S], BF16, tag="ptsb")
                for j in range(NT):
                    pt = ptst.tile([P, P], BF16, tag="tst", name=f"pt_{st}_{j}")
                    nc.tensor.matmul(
                        pt[:], p_sb[:, j * P : (j + 1) * P], identb[:],
                        is_transpose=True,
                    )
                    nc.any.tensor_copy(pt_sb[:, j * P : (j + 1) * P], pt[:])

                po = ppv.tile([P, D], F32, tag="pv")
                for j in range(NT):
                    nc.tensor.matmul(
                        po[:],
                        pt_sb[:, j * P : (j + 1) * P],
                        vh[:, j, :],
                        start=(j == 0),
                        stop=(j == NT - 1),
                    )
                osb = io.tile([P, D], F32, tag="osb", bufs=2)
                nc.scalar.mul(osb[:], po[:], mul=rec[:])
                nc.sync.dma_start(out=out[st * P : (st + 1) * P, :], in_=osb[:])

    nc.compile()
    return nc


def _get_compiled():
    global _COMPILED
    if _COMPILED is None:
        _COMPILED = _build()
    return _COMPILED


def _split16_packed(a):
    hi = a.astype(np.float16)
    lo = (a - hi.astype(np.float32)).astype(np.float16)
    return np.ascontiguousarray(np.stack([hi, lo], axis=1))


def kernel(q, k, v, mask, Wq, Wk, Wv, **_unused):
    import ml_dtypes

    q = np.asarray(q, dtype=np.float32)
    k = np.asarray(k, dtype=np.float32)
    v = np.asarray(v, dtype=np.float32)
    mask = np.asarray(mask)
    wqt = _split16_packed(np.ascontiguousarray(np.asarray(Wq, dtype=np.float32).T))
    wkt = _split16_packed(np.ascontiguousarray(np.asarray(Wk, dtype=np.float32).T))
    wvt = np.ascontiguousarray(
        np.asarray(Wv, dtype=np.float32).T.astype(ml_dtypes.bfloat16)
    )
    maskf = np.where(mask == 0, np.float32(NEG), np.float32(0.0)).astype(
        ml_dtypes.bfloat16
    )

    nc = _get_compiled()

    in_maps = []
    for c in range(N_CORES):
        b, h = divmod(c, 2)
        rows = slice(h * S_LOC, (h + 1) * S_LOC)
        in_maps.append(
            {
                "q": np.ascontiguousarray(q[b, rows]),
                "k": np.ascontiguousarray(k[b, rows]),
                "v": np.ascontiguousarray(v[b, rows]),
                "wqt": wqt,
                "wkt": wkt,
                "wvt": wvt,
                "maskf": maskf[b : b + 1],
            }
        )

    trace = bool(int(os.environ.get("KERNEL_TRACE", "0")))
    res = bass_utils.run_bass_kernel_spmd(
        nc, in_maps, core_ids=list(range(N_CORES)), trace=trace
    )
    if trace:
        kernel.last_exec_time_ns = res.exec_time_ns

    full = np.empty((B, S, D), dtype=np.float32)
    for c in range(N_CORES):
        b, h = divmod(c, 2)
        full[b, h * S_LOC : (h + 1) * S_LOC] = res.results[c]["out"]
    return full


kernel.last_exec_time_ns = None


# revision 34
# speedup vs baseline: 1.0211x; 1.0106x over previous
"""Distributed single-head attention on 8 TRN2 NeuronCores.

Reference computation (fp32):
    qh = q @ Wq.T ; kh = k @ Wk.T ; vh = v @ Wv.T          [B,S,512]
    scores = (qh @ kh.T) * sqrt(4096)                       [B,S,S]
    scores = where(mask==0, -1e9, scores)
    out = softmax(scores, -1) @ vh                          [B,S,512]
with B=4, S=2048, HIDDEN=4096, HEAD=512.

Sharding: 8 cores = (batch b, seq half h); core c handles query rows
[h*1024, (h+1)*1024) of batch b = c//2.  Each core projects only its own
1024 rows of q/k/v; the pair of cores sharing a batch exchanges khT / vh
via an intra-pair AllGather, overlapped with the q projection.

Precision: the softmax is saturated (score std ~1450 after the *64
scale) so the output is ~one-hot @ argmax of the masked scores — score
precision decides correctness.  The q/k projections and QK^T run as
3-pass fp16 hi/lo split matmuls (hi@hi + hi@lo + lo@hi, fp32 PSUM
accumulation): measured 2.2e-7 relative error on silicon — full fp32
quality at 3 PE cycles/row instead of fp32's 4.  The V path only feeds
the post-softmax average, so it runs in bf16 (1 cycle/row, ~3e-3
relative output error).  PE transposes are bit-exact (verified) and put
the contraction dim on partitions for q/k/v and for P^T in PV.
"""

import os
import sys

import numpy as np


def _ensure_path():
    for p in ("/opt/trn_rl_repo", "/opt/pypackages"):
        if os.path.isdir(p) and p not in sys.path:
            sys.path.append(p)


_ensure_path()

from concourse import bacc, masks, tile  # noqa: E402
from concourse import bass_utils  # noqa: E402
from concourse.bass import mybir  # noqa: E402

# S3 upload is unavailable in this container; keep profile artifacts local.
bass_utils.upload_artifacts = lambda tmpdir: tmpdir

F32 = mybir.dt.float32
F16 = mybir.dt.float16
BF16 = mybir.dt.bfloat16

B, S, E, D = 4, 2048, 4096, 512
N_CORES = 8
S_LOC = B * S // N_CORES  # 1024 rows per core
SCALE = float(E) ** 0.5  # 64.0
NEG = -1e9

P = 128
EC = E // P  # 32 contraction chunks for projections
DC = D // P  # 4 head-dim chunks
NT = S // P  # 16 key tiles
ST = S_LOC // P  # 8 query tiles per core

REPLICA_GROUPS = [[0, 1], [2, 3], [4, 5], [6, 7]]

_COMPILED = None


def _build():
    nc = bacc.Bacc("TRN2", target_bir_lowering=False, debug=False, num_devices=N_CORES)

    q_in = nc.dram_tensor("q", [S_LOC, E], F32, kind="ExternalInput").ap()
    k_in = nc.dram_tensor("k", [S_LOC, E], F32, kind="ExternalInput").ap()
    v_in = nc.dram_tensor("v", [S_LOC, E], F32, kind="ExternalInput").ap()
    # W.T hi/lo fp16 pairs packed as [E, 2, D] (host-prepacked)
    wqt = nc.dram_tensor("wqt", [E, 2, D], F16, kind="ExternalInput").ap()
    wkt = nc.dram_tensor("wkt", [E, 2, D], F16, kind="ExternalInput").ap()
    wvt = nc.dram_tensor("wvt", [E, D], BF16, kind="ExternalInput").ap()
    maskf = nc.dram_tensor("maskf", [1, S], BF16, kind="ExternalInput").ap()
    out = nc.dram_tensor("out", [S_LOC, D], F32, kind="ExternalOutput").ap()

    # Internal DRAM bounce buffers for the intra-pair AllGathers.
    # khl_loc packs khT hi at [0] and lo at [1] (fp16).
    khl_loc = nc.dram_tensor("khl_loc", [2, D, S_LOC], F16).ap()
    khl_full = nc.dram_tensor("khl_full", [4, D, S_LOC], F16).ap()
    vh_loc = nc.dram_tensor("vh_loc", [S_LOC, D], BF16).ap()
    vh_full = nc.dram_tensor("vh_full", [S, D], BF16).ap()

    with tile.TileContext(nc) as tc:
        with (
            tc.tile_pool(name="const", bufs=1) as const,
            tc.tile_pool(name="big", bufs=1) as big,
            tc.tile_pool(name="io", bufs=3) as io,
            tc.tile_pool(name="attn", bufs=2) as attn,
            tc.tile_pool(name="small", bufs=4) as small,
            tc.tile_pool(name="pacc", bufs=5, space="PSUM") as pacc,
            tc.tile_pool(name="ptst", bufs=2, space="PSUM") as ptst,
            tc.tile_pool(name="ppv", bufs=1, space="PSUM") as ppv,
        ):
            # ---- constants ----
            ident = const.tile([P, P], F32, tag="ident")
            masks.make_identity(nc, ident[:])
            identb = const.tile([P, P], BF16, tag="identb")
            masks.make_identity(nc, identb[:])
            # maskb[p, t] = maskf[t] for all partitions (0-stride broadcast DMA).
            # bf16: holds only 0.0 / ~-1e9; masked scores underflow to 0 in exp
            # either way, unmasked get exactly +0.0 — no precision impact.
            maskb = const.tile([P, S], BF16, tag="maskb")
            nc.sync.dma_start(out=maskb[:], in_=maskf[:].to_broadcast((P, S)))

            # persistent per-core tensors (fp16 hi/lo for the score path)
            qht_h = big.tile([P, DC, S_LOC], F16, tag="qht_h")
            qht_l = big.tile([P, DC, S_LOC], F16, tag="qht_l")
            kht_h = big.tile([P, DC, S], F16, tag="kht_h")
            kht_l = big.tile([P, DC, S], F16, tag="kht_l")
            vh = big.tile([P, NT, D], BF16, tag="vh")

            def load_sup(x_in, g, e4, tag, bufs=3):
                """Load a [512 rows x 512 cols] block of x as [128, 4, 512]
                (8 contiguous-row DMAs on SWDGE, 1KB runs)."""
                xs = io.tile([P, 4, 512], F32, tag=tag, name=f"{tag}_{g}_{e4}", bufs=bufs)
                for j in range(4):
                    for hh in range(2):
                        nc.gpsimd.dma_start(
                            out=xs[:, j, hh * 256 : (hh + 1) * 256],
                            in_=x_in[
                                g * 512 + j * P : g * 512 + (j + 1) * P,
                                e4 * 512 + hh * 256 : e4 * 512 + (hh + 1) * 256,
                            ],
                        )
                return xs

            def project_hl(x_in, w_in, sink, xtag, xbufs=3, preloaded=None):
                """fp16 hi/lo projection: psum tiles [d 128, s 512] accumulated
                over 32 e-chunks x 3 passes.  One-iteration software pipeline:
                transposes/casts for e run while matmuls for e-1 execute."""
                for g in range(2):
                    accs = [
                        pacc.tile([P, 512], F32, tag="acc", name=f"acc_{g}_{i}")
                        for i in range(4)
                    ]

                    def stage(e, g=g):
                        if e % 4 == 0:
                            if preloaded and (g, e // 4) in preloaded:
                                stage.xs = preloaded.pop((g, e // 4))
                            else:
                                stage.xs = load_sup(x_in, g, e // 4, xtag, bufs=xbufs)
                        whl = io.tile(
                            [P, 2, D], F16, tag="whl", name=f"whl_{g}_{e}", bufs=5
                        )
                        nc.scalar.dma_start(
                            out=whl[:], in_=w_in[e * P : (e + 1) * P, :, :]
                        )
                        el = (e % 4) * P
                        trh = io.tile([P, 512], F16, tag="xTh")
                        trl = io.tile([P, 512], F16, tag="xTl")
                        for j in range(4):
                            pt = ptst.tile([P, P], F32, tag="tst", name=f"t_{g}_{e}_{j}")
                            nc.tensor.matmul(
                                pt[:], stage.xs[:, j, el : el + P], ident[:],
                                is_transpose=True,
                            )
                            nc.any.tensor_copy(trh[:, j * P : (j + 1) * P], pt[:])
                            nc.vector.scalar_tensor_tensor(
                                out=trl[:, j * P : (j + 1) * P],
                                in0=trh[:, j * P : (j + 1) * P], scalar=-1.0,
                                in1=pt[:],
                                op0=mybir.AluOpType.mult, op1=mybir.AluOpType.add,
                            )
                        return whl, trh, trl

                    def matmuls(e, whl, trh, trl):
                        for d in range(4):
                            for p_i, (w_i, xt16) in enumerate(
                                ((0, trh), (0, trl), (1, trh))
                            ):
                                nc.tensor.matmul(
                                    accs[d][:],
                                    whl[:, w_i, d * P : (d + 1) * P],
                                    xt16[:],
                                    start=(e == 0 and p_i == 0),
                                    stop=(e == EC - 1 and p_i == 2),
                                )

                    prev = None
                    for e in range(EC):
                        cur = stage(e)
                        if prev is not None:
                            matmuls(e - 1, *prev)
                        prev = cur
                    matmuls(EC - 1, *prev)

                    for j in range(4):
                        sink(g, j, accs[j][:])

            def split_hl(ps, hi_ap, lo_ap):
                """Evict fp32 psum into fp16 hi + lo."""
                nc.any.tensor_copy(hi_ap, ps)
                nc.vector.scalar_tensor_tensor(
                    out=lo_ap, in0=hi_ap, scalar=-1.0, in1=ps,
                    op0=mybir.AluOpType.mult, op1=mybir.AluOpType.add,
                )

            # ---- v projection (bf16, software-pipelined) -> DRAM bounce ----
            def v_sink(g, j, ps):
                stg = io.tile([P, 512], BF16, tag="postgb", name=f"vs_{g}_{j}", bufs=2)
                nc.any.tensor_copy(stg[:], ps)
                t0 = (g * 4 + j) * P
                nc.sync.dma_start(out=vh_loc[t0 : t0 + P, :], in_=stg[:])

            for g in range(2):
                vaccs = [
                    pacc.tile([P, 512], F32, tag="acc", name=f"vacc_{g}_{i}")
                    for i in range(4)
                ]

                def vstage(e, g=g):
                    if e % 4 == 0:
                        vstage.xs = load_sup(v_in, g, e // 4, "xsupv")
                        vstage.xb = io.tile(
                            [P, 4, 512], BF16, tag="xbsup",
                            name=f"xb_{g}_{e}", bufs=2,
                        )
                        nc.scalar.copy(vstage.xb[:], vstage.xs[:])
                    w = io.tile([P, D], BF16, tag="wloadb", name=f"wv_{g}_{e}", bufs=6)
                    nc.scalar.dma_start(out=w[:], in_=wvt[e * P : (e + 1) * P, :])
                    el = (e % 4) * P
                    trjs = []
                    for j in range(4):
                        pt = ptst.tile([P, P], BF16, tag="tst", name=f"vt_{g}_{e}_{j}")
                        nc.tensor.matmul(
                            pt[:], vstage.xb[:, j, el : el + P], identb[:],
                            is_transpose=True,
                        )
                        trj = io.tile([P, P], BF16, tag="xTj", name=f"vj_{g}_{e}_{j}", bufs=6)
                        nc.any.tensor_copy(trj[:], pt[:])
                        trjs.append(trj)
                    return w, trjs

                def vmatmuls(e, w, trjs, vaccs=vaccs):
                    for j in range(4):
                        nc.tensor.matmul(
                            vaccs[j][:], trjs[j][:], w[:],
                            start=(e == 0), stop=(e == EC - 1),
                        )

                prev = None
                for e in range(EC):
                    cur = vstage(e)
                    if prev is not None:
                        vmatmuls(e - 1, *prev)
                    prev = cur
                vmatmuls(EC - 1, *prev)
                for j in range(4):
                    v_sink(g, j, vaccs[j][:])

            # Next phase's first loads go into the gpsimd queue BEFORE the
            # collective below, so its occupancy can't delay them.
            k_pre = {
                (0, 0): load_sup(k_in, 0, 0, "xsupk"),
                (0, 1): load_sup(k_in, 0, 1, "xsupk"),
            }

            # vh AllGather fires as soon as vh_loc is written (overlaps k+q).
            nc.gpsimd.collective_compute(
                "AllGather",
                mybir.AluOpType.bypass,
                replica_groups=REPLICA_GROUPS,
                ins=[vh_loc.opt()],
                outs=[vh_full.opt()],
            )

            # ---- k projection -> khT hi/lo -> DRAM bounce ----
            def k_sink(g, d, ps):
                sh = io.tile([P, 512], F16, tag="postgh", name=f"ksh_{g}_{d}", bufs=2)
                sl = io.tile([P, 512], F16, tag="postgl", name=f"ksl_{g}_{d}", bufs=2)
                split_hl(ps, sh[:], sl[:])
                nc.sync.dma_start(
                    out=khl_loc[0, d * P : (d + 1) * P, g * 512 : (g + 1) * 512],
                    in_=sh[:],
                )
                nc.sync.dma_start(
                    out=khl_loc[1, d * P : (d + 1) * P, g * 512 : (g + 1) * 512],
                    in_=sl[:],
                )

            project_hl(k_in, wkt, k_sink, "xsupk", preloaded=k_pre)

            q_pre = {
                (0, 0): load_sup(q_in, 0, 0, "xsupq", bufs=2),
                (0, 1): load_sup(q_in, 0, 1, "xsupq", bufs=2),
            }

            nc.gpsimd.collective_compute(
                "AllGather",
                mybir.AluOpType.bypass,
                replica_groups=REPLICA_GROUPS,
                ins=[khl_loc.opt()],
                outs=[khl_full.opt()],
            )

            # ---- q projection -> qht hi/lo (stays in SBUF) ----
            def q_sink(g, d, ps):
                split_hl(
                    ps,
                    qht_h[:, d, g * 512 : (g + 1) * 512],
                    qht_l[:, d, g * 512 : (g + 1) * 512],
                )

            project_hl(q_in, wqt, q_sink, "xsupq", xbufs=2, preloaded=q_pre)

            # ---- gather AG results back to SBUF (split DMAs across engines) ----
            # khl_full[h*2 + {0,1}] = rank-h's khT {hi,lo}.
            for h in range(2):
                for d in range(DC):
                    nc.sync.dma_start(
                        out=kht_h[:, d, h * S_LOC : (h + 1) * S_LOC],
                        in_=khl_full[2 * h, d * P : (d + 1) * P, :],
                    )
                    nc.sync.dma_start(
                        out=kht_l[:, d, h * S_LOC : (h + 1) * S_LOC],
                        in_=khl_full[2 * h + 1, d * P : (d + 1) * P, :],
                    )
            for j in range(NT):
                nc.sync.dma_start(
                    out=vh[:, j, :], in_=vh_full[j * P : (j + 1) * P, :]
                )

            # ---- attention, one 128-query tile at a time ----
            for st in range(ST):
                scs = [
                    pacc.tile([P, 512], F32, tag="acc", name=f"sc_{st}_{i}")
                    for i in range(4)
                ]
                for t4 in range(4):
                    for p_i, (qa, ka) in enumerate(
                        ((qht_h, kht_h), (qht_h, kht_l), (qht_l, kht_h))
                    ):
                        for d in range(4):
                            nc.tensor.matmul(
                                scs[t4][:],
                                qa[:, d, st * P : (st + 1) * P],
                                ka[:, d, t4 * 512 : (t4 + 1) * 512],
                                start=(p_i == 0 and d == 0),
                                stop=(p_i == 2 and d == 3),
                            )
                s_sb = attn.tile([P, S], F32, tag="ssb")
                for t4 in range(4):
                    nc.vector.scalar_tensor_tensor(
                        out=s_sb[:, t4 * 512 : (t4 + 1) * 512],
                        in0=scs[t4][:],
                        scalar=SCALE,
                        in1=maskb[:, t4 * 512 : (t4 + 1) * 512],
                        op0=mybir.AluOpType.mult,
                        op1=mybir.AluOpType.add,
                    )
                cmax = small.tile([P, 4], F32, tag="cmax")
                for t4 in range(4):
                    nc.vector.tensor_reduce(
                        cmax[:, t4 : t4 + 1], s_sb[:, t4 * 512 : (t4 + 1) * 512],
                        axis=mybir.AxisListType.X, op=mybir.AluOpType.max,
                    )
                nmax = small.tile([P, 1], F32, tag="nmax")
                nc.vector.tensor_reduce(
                    nmax[:], cmax[:],
                    axis=mybir.AxisListType.X, op=mybir.AluOpType.max, negate=True,
                )
                p_sb = attn.tile([P, S], BF16, tag="psb")
                rs4 = small.tile([P, 4], F32, tag="rs4")
                for t4 in range(4):
                    nc.scalar.activation(
                        p_sb[:, t4 * 512 : (t4 + 1) * 512],
                        s_sb[:, t4 * 512 : (t4 + 1) * 512],
                        mybir.ActivationFunctionType.Exp,
                        bias=nmax[:], scale=1.0,
                        accum_out=rs4[:, t4 : t4 + 1],
                    )
                rsum = small.tile([P, 1], F32, tag="rsum")
                nc.vector.tensor_reduce(
                    rsum[:], rs4[:], axis=mybir.AxisListType.X, op=mybir.AluOpType.add,
                )
                rec = small.tile([P, 1], F32, tag="rec")
                nc.vector.reciprocal(rec[:], rsum[:])

                pt_sb = attn.tile([P, S], BF16, tag="ptsb")
                for j in range(NT):
                    pt = ptst.tile([P, P], BF16, tag="tst", name=f"pt_{st}_{j}")
                    nc.tensor.matmul(
                        pt[:], p_sb[:, j * P : (j + 1) * P], identb[:],
                        is_transpose=True,
                    )
                    nc.any.tensor_copy(pt_sb[:, j * P : (j + 1) * P], pt[:])

                po = ppv.tile([P, D], F32, tag="pv")
                for j in range(NT):
                    nc.tensor.matmul(
                        po[:],
                        pt_sb[:, j * P : (j + 1) * P],
                        vh[:, j, :],
                        start=(j == 0),
                        stop=(j == NT - 1),
                    )
                osb = io.tile([P, D], F32, tag="osb", bufs=2)
                nc.scalar.mul(osb[:], po[:], mul=rec[:])
                nc.sync.dma_start(out=out[st * P : (st + 1) * P, :], in_=osb[:])

    nc.compile()
    return nc


def _get_compiled():
    global _COMPILED
    if _COMPILED is None:
        _COMPILED = _build()
    return _COMPILED


def _split16_packed(a):
    hi = a.astype(np.float16)
    lo = (a - hi.astype(np.float32)).astype(np.float16)
    return np.ascontiguousarray(np.stack([hi, lo], axis=1))


def kernel(q, k, v, mask, Wq, Wk, Wv, **_unused):
    import ml_dtypes

    q = np.asarray(q, dtype=np.float32)
    k = np.asarray(k, dtype=np.float32)
    v = np.asarray(v, dtype=np.float32)
    mask = np.asarray(mask)
    wqt = _split16_packed(np.ascontiguousarray(np.asarray(Wq, dtype=np.float32).T))
    wkt = _split16_packed(np.ascontiguousarray(np.asarray(Wk, dtype=np.float32).T))
    wvt = np.ascontiguousarray(
        np.asarray(Wv, dtype=np.float32).T.astype(ml_dtypes.bfloat16)
    )
    maskf = np.where(mask == 0, np.float32(NEG), np.float32(0.0)).astype(
        ml_dtypes.bfloat16
    )

    nc = _get_compiled()

    in_maps = []
    for c in range(N_CORES):
        b, h = divmod(c, 2)
        rows = slice(h * S_LOC, (h + 1) * S_LOC)
        in_maps.append(
            {
                "q": np.ascontiguousarray(q[b, rows]),
                "k": np.ascontiguousarray(k[b, rows]),
                "v": np.ascontiguousarray(v[b, rows]),
                "wqt": wqt,
                "wkt": wkt,
                "wvt": wvt,
                "maskf": maskf[b : b + 1],
            }
        )

    trace = bool(int(os.environ.get("KERNEL_TRACE", "0")))
    res = bass_utils.run_bass_kernel_spmd(
        nc, in_maps, core_ids=list(range(N_CORES)), trace=trace
    )
    if trace:
        kernel.last_exec_time_ns = res.exec_time_ns

    full = np.empty((B, S, D), dtype=np.float32)
    for c in range(N_CORES):
        b, h = divmod(c, 2)
        full[b, h * S_LOC : (h + 1) * S_LOC] = res.results[c]["out"]
    return full


kernel.last_exec_time_ns = None
